# revision 1
# baseline (speedup 1.0000x reference)
"""Trainium2 Bass kernel for nn_MCLoss (scatter_memory forward).

Computes logits = inputs @ memory.T  ([4096, 2048] @ [2048, 50000] -> [4096, 50000] f32).

Strategy (tensor-parallel, per sharding hint): the memory bank is sharded
row-wise across 8 NeuronCores (6250 identity rows each, zero-padded to 6272 =
49*128 columns of the per-core logits slice). Each core computes its
[4096, 6272] slice of the logits with a tiled PE matmul; the host concatenates
the 8 slices and drops the padding.

Device kernel (per core, identical SPMD program):
  - lhs (stationary operand tiles): inputs pre-transposed on host into
    [128, 32, 16, 128] tile layout, dtype float32r.  A[p, m, k, j] =
    inputs[m*128 + j, k*128 + p], so lhsT tile (k, m) = A[:, m, k, :] is a
    [K=128, M=128] tile with the contraction dim on partitions.
  - rhs (moving operand): memory shard transposed on host to [2048, 6272]
    (memT[d, c] = memory[c, d]), dtype float32r.
  - float32r runs the 128x128 PE at 1 cycle/row for moving dims >= 256
    (bf16-class throughput) while keeping ~12-13 mantissa bits => rel err
    ~1.5e-4 on unit-norm rows, 16x better than bf16 at the same speed.
  - Loop nest: 5 column groups of width 1280 (paired with 3 PSUM banks,
    double-buffered), rhs group tiles resident in SBUF and reused by all 32
    m-tiles; per (group, m): one 1 MB lhs DMA, then 16 k-tiles x 3 bank
    slices of accumulating matmuls; PSUM evicted via VectorE copy to SBUF
    and DMA'd straight into the final [4096, 6272] layout.
"""
import numpy as np

import concourse.bass as bass
import concourse.mybir as mybir
import concourse.tile as tile
from concourse import bacc
from concourse.bass_utils import run_bass_kernel_spmd

P = 128
B = 4096          # rows of inputs
D = 2048          # features (contraction)
C = 50000         # memory rows (classes)
N_CORES = 8
N_SHARD = C // N_CORES          # 6250
N_PAD = 6272                    # 49 * 128, per-core padded logits width
CW = 1280                       # column-group width (3 PSUM banks: 512+512+256)
MT = B // P                     # 32
KT = D // P                     # 16

_NC_CACHE = {}


def _bank_slices(w):
    out, c = [], 0
    while c < w:
        s = min(512, w - c)
        out.append((c, s))
        c += s
    return out


def _build():
    if "nc" in _NC_CACHE:
        return _NC_CACHE["nc"]
    dt_in = mybir.dt.float32r
    nc = bacc.Bacc("TRN2", target_bir_lowering=False, debug=False)
    lhs = nc.dram_tensor("lhs", [P, MT, KT, P], dt_in, kind="ExternalInput")
    rhs = nc.dram_tensor("rhs", [D, N_PAD], dt_in, kind="ExternalInput")
    out = nc.dram_tensor("out", [B, N_PAD], mybir.dt.float32, kind="ExternalOutput")
    rhs_r = rhs[:].rearrange("(k p) c -> p k c", p=P)

    groups, c0 = [], 0
    while c0 < N_PAD:
        w = min(CW, N_PAD - c0)
        groups.append((c0, w))
        c0 += w

    with tile.TileContext(nc) as tc:
        with (
            tc.tile_pool(name="rhsp", bufs=2) as rhsp,
            tc.tile_pool(name="lhsp", bufs=4) as lhsp,
            tc.tile_pool(name="outp", bufs=2) as outp,
            tc.tile_pool(name="psump", bufs=2, space="PSUM") as psump,
        ):
            for c0, w in groups:
                rt = rhsp.tile([P, KT, w], dt_in, tag="rhs")
                nc.sync.dma_start(out=rt[:], in_=rhs_r[:, :, c0 : c0 + w])
                for m in range(MT):
                    lt = lhsp.tile([P, KT, P], dt_in, tag="lhs")
                    nc.sync.dma_start(out=lt[:], in_=lhs[:, m, :, :])
                    ps = psump.tile([P, w], mybir.dt.float32, tag="ps")
                    for k in range(KT):
                        for n0, nw in _bank_slices(w):
                            nc.tensor.matmul(
                                ps[:, n0 : n0 + nw],
                                lhsT=lt[:, k, :],
                                rhs=rt[:, k, n0 : n0 + nw],
                                start=(k == 0),
                                stop=(k == KT - 1),
                            )
                    ot = outp.tile([P, w], mybir.dt.float32, tag="out")
                    nc.vector.tensor_copy(out=ot[:], in_=ps[:])
                    nc.sync.dma_start(
                        out=out[m * P : (m + 1) * P, c0 : c0 + w], in_=ot[:]
                    )
    nc.compile()
    _NC_CACHE["nc"] = nc
    return nc


def _prep_inputs(inputs, memory):
    inputs = np.ascontiguousarray(np.asarray(inputs, dtype=np.float32))
    memory = np.asarray(memory, dtype=np.float32)
    # lhs tile layout: A[p, m, k, j] = inputs[m*128 + j, k*128 + p]
    lhs_np = np.ascontiguousarray(
        inputs.reshape(MT, P, KT, P).transpose(3, 0, 2, 1)
    )
    # per-core rhs: memT shard [D, N_PAD] with zero padding
    rhs_all = np.zeros((N_CORES, D, N_PAD), np.float32)
    rhs_all[:, :, :N_SHARD] = memory.reshape(N_CORES, N_SHARD, D).transpose(0, 2, 1)
    return lhs_np, rhs_all


def kernel(inputs, targets, memory):
    """Full-input entry point: returns logits [4096, 50000] float32."""
    nc = _build()
    lhs_np, rhs_all = _prep_inputs(inputs, memory)
    in_maps = [{"lhs": lhs_np, "rhs": rhs_all[c]} for c in range(N_CORES)]
    res = run_bass_kernel_spmd(nc, in_maps, core_ids=list(range(N_CORES)))
    logits = np.concatenate(
        [res.results[c]["out"][:, :N_SHARD] for c in range(N_CORES)], axis=1
    )
    return np.ascontiguousarray(logits)
